# revision 10
# baseline (speedup 1.0000x reference)
"""Trainium2 Bass kernel for the MemoryModule problem.

Per batch element b (8 of them, one per NeuronCore):
    mk = memory_keys[:, b]  viewed as (Ck=128, M=8192)   [M = T*H*W]
    mv = memory_values[:, b] viewed as (Cv=512, M)
    qk = query_key[b]       viewed as (Ck=128, N=1024)   [N = H*W]
    S  = qk^T @ mk          (N, M)
    P  = softmax(S, axis=-1)
    mem = (P @ mv^T)^T      (Cv, N)
    out[b] = concat([query_value[b], mem], channel axis)

Device dataflow (all transposes done on host / by layout, none on chip):
    - S^T computed directly: S^T tile (128 m, n) = matmul(lhsT=mk_tile, rhs=qk)
      with fp16 inputs.
    - exp on ScalarE (no max subtraction: |S| <~ 70, exp fits fp32/bf16 range),
      written as bf16 P^T tiles.
    - PV: matmul(lhsT=P^T chunk (m,128n), rhs=mv^T tile (m,512c)) in bf16,
      accumulated over the 64 m tiles in PSUM.
    - softmax denominator: VectorE accumulates the P^T tiles into an SBUF
      fp32 accumulator; per half, 4 tiny PE matmuls against a ones vector
      reduce the 128 partition-partials per n column (output partition = n,
      which also transposes the denominator into the orientation the
      normalize step needs).
    - normalize with reciprocal + per-partition scalar multiply, DMA out
      as mem^T (N, Cv); host transposes back.

Key scheduling insight (measured): interleaving single-shot S^T matmuls
with the PV psum-accumulation stream costs ~+43ns on EVERY PE instruction
(pipeline disturbance at psum group/mode switches). Batching the S^T
matmuls in groups of 4 (ST_BATCH) between 16-instr PV phases recovers it
(253 -> 199 ns/instr in microbenches). n is processed in halves of 512 so
PSUM fits: 3 S^T banks + 4 PV accumulators + 1 denominator bank = 8.
"""

import os

import numpy as np
import ml_dtypes

T, B, Ck, Cv, H, W = 8, 8, 128, 512, 32, 32
HW = H * W            # 1024  (n dimension)
M = T * HW            # 8192  (memory / contraction dimension)
MT = M // 128         # 64 m-tiles
NQ = 2                # process n in halves
NQS = HW // NQ        # 512 columns of S^T per half
NCH = NQS // 128      # 4 PV accumulators per half
N_CORES = 8

QK_MODE = os.environ.get("KERNEL_QK_MODE", "f16")
# m-tiles per S^T batch (PE psum-mode-switch amortization)
STB = int(os.environ.get("KERNEL_STB", "4"))
NG = MT // STB        # st/pv groups per half
# groups of lookahead for st/exp emission relative to pv consumption
LEAD = int(os.environ.get("KERNEL_LEAD", "2"))
PT_BUFS = int(os.environ.get("KERNEL_PT_BUFS", "16"))
# >1: repeat the full compute (incl. input DMAs) inside one NEFF via a
# hardware For_i loop, for HW timing via wall-clock deltas.
LOOP = int(os.environ.get("KERNEL_LOOP", "1"))
# timing diagnostics: "full" | "dma" (loop only DMAs) | "compute" (DMAs
# hoisted out of the loop, loop only compute)
MODE = os.environ.get("KERNEL_MODE", "full")
# timing bisection knobs (numerics become wrong; timing-only)
DUMMYPT = os.environ.get("KERNEL_DUMMYPT", "0") == "1"  # pv reads constant tile
NOACC = os.environ.get("KERNEL_NOACC", "0") == "1"      # skip DVE acc chain

_CACHE = {}
LAST_RESULTS = None


def _build_nc(qk_mode, loop=1, mode="full"):
    import concourse.tile as tile
    import concourse.mybir as mybir
    from concourse import bacc

    f32 = mybir.dt.float32
    bf16 = mybir.dt.bfloat16
    f16 = mybir.dt.float16
    qk_dt = {"f16": f16, "f32r": f32, "f32": f32}[qk_mode]

    nc = bacc.Bacc()

    qk_d = nc.dram_tensor("qk", [Ck, HW], qk_dt, kind="ExternalInput")
    mk_d = nc.dram_tensor("mk", [Ck, M], qk_dt, kind="ExternalInput")
    mv_d = nc.dram_tensor("mv", [M, Cv], bf16, kind="ExternalInput")
    out_d = nc.dram_tensor("out", [Cv, HW], f32, kind="ExternalOutput")

    mv_tiled = mv_d.rearrange("(mt p) c -> mt p c", p=128)  # (64, 128, 512)

    Exp = mybir.ActivationFunctionType.Exp
    Copy = mybir.ActivationFunctionType.Copy
    AluOp = mybir.AluOpType

    def emit_dma(nc, tc, big):
        qk_sb = big.tile([Ck, HW], qk_dt, tag="qk_sb", name="qk_sb")
        nc.sync.dma_start(qk_sb[:], qk_d[:])
        mk_sb = big.tile([Ck, M], qk_dt, tag="mk_sb", name="mk_sb")
        # split so the first S^T matmuls don't wait for the whole tensor
        for i in range(8):
            nc.sync.dma_start(
                mk_sb[:, i * HW : (i + 1) * HW], mk_d[:, i * HW : (i + 1) * HW]
            )
        mv_sb = big.tile([128, MT, Cv], bf16, tag="mv_sb", name="mv_sb")
        # the 8MB mv stream rides the ACT hwdge queue so it doesn't serialize
        # behind qk/mk (and ahead of the out stores) on the SP queue
        for m in range(MT):
            nc.scalar.dma_start(mv_sb[:, m], mv_tiled[m])
        return qk_sb, mk_sb, mv_sb

    def body(nc, tc, big, ptp, accp, outp, smallp, stp, pvp, tiles):
        qk_sb, mk_sb, mv_sb = tiles
        ones_sb = big.tile([128, 1], f32, tag="ones_sb", name="ones_sb")
        nc.vector.memset(ones_sb[:], 1.0)
        ones_row = big.tile([1, 128], f32, tag="ones_row", name="ones_row")
        nc.vector.memset(ones_row[:], 1.0)
        dummy_pt = None
        if DUMMYPT:
            dummy_pt = big.tile([128, NQS], bf16, tag="dummy_pt", name="dummy_pt")
            nc.vector.memset(dummy_pt[:], 0.001)

        def mm_cast(ap):
            if qk_mode == "f32r":
                return ap.bitcast(mybir.dt.float32r)
            return ap

        class Half:
            def __init__(self, q):
                self.q = q
                self.sts = {}
                self.pts = {}

            def emit_st_exp_group(self, g):
                q = self.q
                ms = range(g * STB, (g + 1) * STB)
                # batch all st matmuls, then all exps, to keep the PE's
                # psum-mode switches at group granularity
                for m in ms:
                    st = stp.tile(
                        [128, NQS], f32, tag="st", name=f"st_q{q}_m{m}"
                    )
                    nc.tensor.matmul(
                        st[:],
                        mm_cast(mk_sb[:, m * 128 : (m + 1) * 128]),
                        mm_cast(qk_sb[:, q * NQS : (q + 1) * NQS]),
                        start=True,
                        stop=True,
                    )
                    self.sts[m] = st
                for m in ms:
                    pt = ptp.tile(
                        [128, NQS], bf16, tag="pt", name=f"pt_q{q}_m{m}"
                    )
                    nc.scalar.activation(pt[:], self.sts.pop(m)[:], Exp)
                    self.pts[m] = pt

            def emit_acc(self, m):
                # VectorE: accumulate exp tiles for the softmax denominator
                if NOACC:
                    if m == 0:
                        nc.vector.memset(self.acc[:], 1.0)
                    return
                if m == 0:
                    nc.vector.tensor_copy(self.acc[:], self.pts[m][:])
                else:
                    nc.vector.tensor_tensor(
                        self.acc[:], self.acc[:], self.pts[m][:], AluOp.add
                    )

            def emit_pv_group(self, g):
                # stationary = mv chunk (LDW is pipelined/hidden), moving =
                # the P^T pool tile (small SBUF working set -> full-rate
                # moving stream). Output orientation becomes (c-chunk, n).
                for m in range(g * STB, (g + 1) * STB):
                    ptm = dummy_pt if DUMMYPT else self.pts[m]
                    for j in range(NCH):
                        nc.tensor.matmul(
                            self.pv[j][:],
                            mv_sb[:, m, j * 128 : (j + 1) * 128],
                            ptm[:],
                            start=(m == 0),
                            stop=(m == MT - 1),
                        )
                    self.emit_acc(m)
                    del self.pts[m]

            def prologue(self):
                q = self.q
                self.pv = [
                    pvp.tile([128, NQS], f32, tag=f"pv{j}", name=f"pv_q{q}_{j}")
                    for j in range(NCH)
                ]
                self.acc = accp.tile([128, NQS], f32, tag="acc", name=f"acc_q{q}")
                for g in range(LEAD):
                    self.emit_st_exp_group(g)

            def mloop(self):
                for g in range(NG):
                    self.emit_pv_group(g)
                    if g + LEAD < NG:
                        self.emit_st_exp_group(g + LEAD)

            def epilogue(self):
                q = self.q
                # denominator as a row: dn_row[0, n] = sum_p acc[p, n] via a
                # ones-column matmul, then reciprocal and broadcast across
                # the 128 partitions with a second ones matmul. psum tiles
                # come from the S^T pool (drained by now).
                dn = stp.tile([128, NQS], f32, tag="st", name=f"dn_q{q}")
                nc.tensor.matmul(
                    dn[0:1, :], ones_sb[:], self.acc[:], start=True, stop=True
                )
                recip_row = smallp.tile(
                    [1, NQS], f32, tag="recip", name=f"recip_q{q}"
                )
                nc.vector.reciprocal(recip_row[:], dn[0:1, :])
                rb_ps = stp.tile([128, NQS], f32, tag="st", name=f"rb_ps_q{q}")
                nc.tensor.matmul(
                    rb_ps[:], ones_row[0:1, :], recip_row[:], start=True, stop=True
                )
                rb = smallp.tile([128, NQS], f32, tag="rb", name=f"rb_q{q}")
                nc.vector.tensor_copy(rb[:], rb_ps[:])
                for j in range(NCH):
                    o = outp.tile([128, NQS], f32, tag="o", name=f"o_q{q}_{j}")
                    # split the tail normalize across DVE and GPSIMD so the
                    # final PSUM evacuation overlaps
                    nc.vector.tensor_tensor(o[:], self.pv[j][:], rb[:], AluOp.mult)
                    nc.sync.dma_start(
                        out_d[j * 128 : (j + 1) * 128, q * NQS : (q + 1) * NQS],
                        o[:],
                    )

        # interleave: emit the next half's prologue before this half's
        # epilogue so PE isn't FIFO-blocked behind the denominator matmuls
        # (which wait on the DVE accumulation tail)
        halves = [Half(q) for q in range(NQ)]
        halves[0].prologue()
        for q in range(NQ):
            halves[q].mloop()
            if q + 1 < NQ:
                halves[q + 1].prologue()
            halves[q].epilogue()

    with tile.TileContext(nc) as tc:
        with (
            tc.tile_pool(name="big", bufs=1) as big,
            tc.tile_pool(name="ptp", bufs=PT_BUFS) as ptp,
            tc.tile_pool(name="accp", bufs=2) as accp,
            tc.tile_pool(name="outp", bufs=6) as outp,
            tc.tile_pool(name="smallp", bufs=4) as smallp,
            tc.tile_pool(name="stp", bufs=4, space="PSUM") as stp,
            tc.tile_pool(name="pvp", bufs=1, space="PSUM") as pvp,
        ):
            if mode == "compute" and loop > 1:
                tiles = emit_dma(nc, tc, big)
                with tc.For_i(0, loop, 1):
                    body(nc, tc, big, ptp, accp, outp, smallp, stp, pvp, tiles)
            elif mode == "dma" and loop > 1:
                with tc.For_i(0, loop, 1):
                    emit_dma(nc, tc, big)
                    for j in range(8):
                        o = outp.tile([128, Cv], f32, tag="o", name=f"o_{j}")
                        nc.vector.memset(o[:], float(j))
                        nc.sync.dma_start(out_d[j * 128 : (j + 1) * 128, :], o[:])
            else:
                loop_ctx = tc.For_i(0, loop, 1) if loop > 1 else None
                with (loop_ctx if loop_ctx is not None else _null()):
                    tiles = emit_dma(nc, tc, big)
                    body(nc, tc, big, ptp, accp, outp, smallp, stp, pvp, tiles)

    nc.finalize()
    return nc


class _null:
    def __enter__(self):
        return None

    def __exit__(self, *a):
        return False


def _get_nc():
    key = ("nc", QK_MODE, LOOP, MODE, STB, LEAD, PT_BUFS, DUMMYPT, NOACC)
    if key not in _CACHE:
        _CACHE[key] = _build_nc(QK_MODE, LOOP, MODE)
    return _CACHE[key]


def _prep_core_inputs(memory_keys, memory_values, query_key, b):
    np_qk_dt = np.float16 if QK_MODE == "f16" else np.float32
    # astype on the transposed view fuses cast+copy in one pass
    qk = query_key[b].reshape(Ck, HW).astype(np_qk_dt)
    mk = memory_keys[:, b].transpose(1, 0, 2, 3).astype(np_qk_dt).reshape(Ck, M)
    mv = (
        memory_values[:, b]
        .transpose(0, 2, 3, 1)
        .astype(ml_dtypes.bfloat16)
        .reshape(M, Cv)
    )
    return {"qk": qk, "mk": mk, "mv": mv}


_RUNNER = {}


def _get_runner():
    """Build the sharded PJRT callable once and reuse it — the generic
    run_bass_kernel_spmd path re-traces jax.jit on every call (~2 s)."""
    if "r" not in _RUNNER:
        import jax
        from jax.sharding import Mesh, PartitionSpec, NamedSharding
        from jax.experimental.shard_map import shard_map

        import concourse.mybir as mybir
        from concourse import bass2jax
        from concourse.bass2jax import _bass_exec_p, install_neuronx_cc_hook

        nc = _get_nc()
        install_neuronx_cc_hook()
        pname = nc.partition_id_tensor.name if nc.partition_id_tensor else None
        in_names, out_names, out_avals = [], [], []
        for alloc in nc.m.functions[0].allocations:
            if not isinstance(alloc, mybir.MemoryLocationSet):
                continue
            name = alloc.memorylocations[0].name
            if alloc.kind == "ExternalInput":
                if name != pname:
                    in_names.append(name)
            elif alloc.kind == "ExternalOutput":
                out_names.append(name)
                out_avals.append(
                    jax.core.ShapedArray(
                        tuple(alloc.tensor_shape), mybir.dt.np(alloc.dtype)
                    )
                )
        n_params = len(in_names)
        all_in = list(in_names) + list(out_names) + ([pname] if pname else [])

        def _body(*args):
            operands = list(args)
            if pname is not None:
                operands.append(bass2jax.partition_id_tensor())
            return tuple(
                _bass_exec_p.bind(
                    *operands,
                    out_avals=tuple(out_avals),
                    in_names=tuple(all_in),
                    out_names=tuple(out_names),
                    lowering_input_output_aliases=(),
                    sim_require_finite=True,
                    sim_require_nnan=True,
                    nc=nc,
                )
            )

        mesh = Mesh(np.asarray(jax.devices()[:N_CORES]), ("core",))
        n_outs = len(out_names)
        sharded = jax.jit(
            shard_map(
                _body,
                mesh=mesh,
                in_specs=(PartitionSpec("core"),) * (n_params + n_outs),
                out_specs=(PartitionSpec("core"),) * n_outs,
                check_rep=False,
            ),
            keep_unused=True,
        )
        sh = NamedSharding(mesh, PartitionSpec("core"))
        zeros = [
            jax.device_put(
                np.zeros((N_CORES * a.shape[0], *a.shape[1:]), a.dtype), sh
            )
            for a in out_avals
        ]
        _RUNNER["r"] = (sharded, sh, in_names, zeros)
    return _RUNNER["r"]


def kernel(memory_keys, memory_values, query_key, query_value):
    global LAST_RESULTS
    memory_keys = np.asarray(memory_keys, dtype=np.float32)
    memory_values = np.asarray(memory_values, dtype=np.float32)
    query_key = np.asarray(query_key, dtype=np.float32)
    query_value = np.asarray(query_value, dtype=np.float32)

    in_maps = [
        _prep_core_inputs(memory_keys, memory_values, query_key, b)
        for b in range(N_CORES)
    ]
    try:
        import jax

        sharded, sh, in_names, zeros = _get_runner()
        dev_in = [
            jax.device_put(
                np.concatenate([in_maps[c][n] for c in range(N_CORES)], 0), sh
            )
            for n in in_names
        ]
        outs = sharded(*dev_in, *zeros)
        out_full = np.asarray(outs[0]).reshape(N_CORES, Cv, HW)
    except Exception:
        # conservative fallback: the generic (slower per call) exec path
        from concourse.bass_utils import run_bass_kernel_spmd

        res = run_bass_kernel_spmd(
            _get_nc(), in_maps, core_ids=list(range(N_CORES))
        )
        LAST_RESULTS = res
        out_full = np.stack([res.results[b]["out"] for b in range(N_CORES)])  # (B, Cv, HW)

    mem = out_full.reshape(N_CORES, Cv, H, W).astype(np.float32)
    return np.concatenate([query_value, mem], axis=1)


# revision 11
# speedup vs baseline: 1.0761x; 1.0761x over previous
"""Trainium2 Bass kernel for the MemoryModule problem.

Per batch element b (8 of them, one per NeuronCore):
    mk = memory_keys[:, b]  viewed as (Ck=128, M=8192)   [M = T*H*W]
    mv = memory_values[:, b] viewed as (Cv=512, M)
    qk = query_key[b]       viewed as (Ck=128, N=1024)   [N = H*W]
    S  = qk^T @ mk          (N, M)
    P  = softmax(S, axis=-1)
    mem = (P @ mv^T)^T      (Cv, N)
    out[b] = concat([query_value[b], mem], channel axis)

Device dataflow (all transposes done on host / by layout, none on chip):
    - S^T computed directly: S^T tile (128 m, n) = matmul(lhsT=mk_tile, rhs=qk)
      with fp16 inputs.
    - exp on ScalarE (no max subtraction: |S| <~ 70, exp fits fp32/bf16 range),
      written as bf16 P^T tiles.
    - PV: matmul(lhsT=P^T chunk (m,128n), rhs=mv^T tile (m,512c)) in bf16,
      accumulated over the 64 m tiles in PSUM.
    - softmax denominator: VectorE accumulates the P^T tiles into an SBUF
      fp32 accumulator; per half, 4 tiny PE matmuls against a ones vector
      reduce the 128 partition-partials per n column (output partition = n,
      which also transposes the denominator into the orientation the
      normalize step needs).
    - normalize with reciprocal + per-partition scalar multiply, DMA out
      as mem^T (N, Cv); host transposes back.

Key scheduling insight (measured): interleaving single-shot S^T matmuls
with the PV psum-accumulation stream costs ~+43ns on EVERY PE instruction
(pipeline disturbance at psum group/mode switches). Batching the S^T
matmuls in groups of 4 (ST_BATCH) between 16-instr PV phases recovers it
(253 -> 199 ns/instr in microbenches). n is processed in halves of 512 so
PSUM fits: 3 S^T banks + 4 PV accumulators + 1 denominator bank = 8.
"""

import os

import numpy as np
import ml_dtypes

T, B, Ck, Cv, H, W = 8, 8, 128, 512, 32, 32
HW = H * W            # 1024  (n dimension)
M = T * HW            # 8192  (memory / contraction dimension)
MT = M // 128         # 64 m-tiles
NQ = 2                # process n in halves
NQS = HW // NQ        # 512 columns of S^T per half
NCH = NQS // 128      # 4 PV accumulators per half
N_CORES = 8

QK_MODE = os.environ.get("KERNEL_QK_MODE", "f16")
# m-tiles per S^T batch (PE psum-mode-switch amortization)
STB = int(os.environ.get("KERNEL_STB", "4"))
NG = MT // STB        # st/pv groups per half
# groups of lookahead for st/exp emission relative to pv consumption
LEAD = int(os.environ.get("KERNEL_LEAD", "2"))
PT_BUFS = int(os.environ.get("KERNEL_PT_BUFS", "16"))
# >1: repeat the full compute (incl. input DMAs) inside one NEFF via a
# hardware For_i loop, for HW timing via wall-clock deltas.
LOOP = int(os.environ.get("KERNEL_LOOP", "1"))
# timing diagnostics: "full" | "dma" (loop only DMAs) | "compute" (DMAs
# hoisted out of the loop, loop only compute)
MODE = os.environ.get("KERNEL_MODE", "full")
# timing bisection knobs (numerics become wrong; timing-only)
DUMMYPT = os.environ.get("KERNEL_DUMMYPT", "0") == "1"  # pv reads constant tile
NOACC = os.environ.get("KERNEL_NOACC", "0") == "1"      # skip DVE acc chain

_CACHE = {}
LAST_RESULTS = None


def _build_nc(qk_mode, loop=1, mode="full"):
    import concourse.tile as tile
    import concourse.mybir as mybir
    from concourse import bacc

    f32 = mybir.dt.float32
    bf16 = mybir.dt.bfloat16
    f16 = mybir.dt.float16
    qk_dt = {"f16": f16, "f32r": f32, "f32": f32}[qk_mode]

    nc = bacc.Bacc()

    qk_d = nc.dram_tensor("qk", [Ck, HW], qk_dt, kind="ExternalInput")
    mk_d = nc.dram_tensor("mk", [Ck, M], qk_dt, kind="ExternalInput")
    mv_d = nc.dram_tensor("mv", [M, Cv], bf16, kind="ExternalInput")
    out_d = nc.dram_tensor("out", [HW, Cv], f32, kind="ExternalOutput")

    mv_tiled = mv_d.rearrange("(mt p) c -> mt p c", p=128)  # (64, 128, 512)

    Exp = mybir.ActivationFunctionType.Exp
    Copy = mybir.ActivationFunctionType.Copy
    AluOp = mybir.AluOpType

    def emit_dma(nc, tc, big):
        qk_sb = big.tile([Ck, HW], qk_dt, tag="qk_sb", name="qk_sb")
        nc.sync.dma_start(qk_sb[:], qk_d[:])
        mk_sb = big.tile([Ck, M], qk_dt, tag="mk_sb", name="mk_sb")
        # split so the first S^T matmuls don't wait for the whole tensor
        for i in range(8):
            nc.sync.dma_start(
                mk_sb[:, i * HW : (i + 1) * HW], mk_d[:, i * HW : (i + 1) * HW]
            )
        mv_sb = big.tile([128, MT, Cv], bf16, tag="mv_sb", name="mv_sb")
        # the 8MB mv stream rides the ACT hwdge queue so it doesn't serialize
        # behind qk/mk (and ahead of the out stores) on the SP queue
        for m in range(MT):
            nc.scalar.dma_start(mv_sb[:, m], mv_tiled[m])
        return qk_sb, mk_sb, mv_sb

    def body(nc, tc, big, ptp, accp, outp, smallp, stp, pvp, tiles):
        qk_sb, mk_sb, mv_sb = tiles
        ones_sb = big.tile([128, 1], f32, tag="ones_sb", name="ones_sb")
        nc.vector.memset(ones_sb[:], 1.0)
        dummy_pt = None
        if DUMMYPT:
            dummy_pt = big.tile([128, NQS], bf16, tag="dummy_pt", name="dummy_pt")
            nc.vector.memset(dummy_pt[:], 0.001)

        def mm_cast(ap):
            if qk_mode == "f32r":
                return ap.bitcast(mybir.dt.float32r)
            return ap

        class Half:
            def __init__(self, q):
                self.q = q
                self.sts = {}
                self.pts = {}

            def emit_st_exp_group(self, g):
                q = self.q
                ms = range(g * STB, (g + 1) * STB)
                # batch all st matmuls, then all exps, to keep the PE's
                # psum-mode switches at group granularity
                for m in ms:
                    st = stp.tile(
                        [128, NQS], f32, tag="st", name=f"st_q{q}_m{m}"
                    )
                    nc.tensor.matmul(
                        st[:],
                        mm_cast(mk_sb[:, m * 128 : (m + 1) * 128]),
                        mm_cast(qk_sb[:, q * NQS : (q + 1) * NQS]),
                        start=True,
                        stop=True,
                    )
                    self.sts[m] = st
                for m in ms:
                    pt = ptp.tile(
                        [128, NQS], bf16, tag="pt", name=f"pt_q{q}_m{m}"
                    )
                    nc.scalar.activation(pt[:], self.sts.pop(m)[:], Exp)
                    self.pts[m] = pt

            def emit_acc(self, m):
                # VectorE: accumulate exp tiles for the softmax denominator
                if NOACC:
                    if m == 0:
                        nc.vector.memset(self.acc[:], 1.0)
                    return
                if m == 0:
                    nc.vector.tensor_copy(self.acc[:], self.pts[m][:])
                else:
                    nc.vector.tensor_tensor(
                        self.acc[:], self.acc[:], self.pts[m][:], AluOp.add
                    )

            def emit_pv_group(self, g):
                for m in range(g * STB, (g + 1) * STB):
                    ptm = dummy_pt if DUMMYPT else self.pts[m]
                    for j in range(NCH):
                        nc.tensor.matmul(
                            self.pv[j][:],
                            ptm[:, j * 128 : (j + 1) * 128],
                            mv_sb[:, m],
                            start=(m == 0),
                            stop=(m == MT - 1),
                        )
                    self.emit_acc(m)
                    del self.pts[m]

            def prologue(self):
                q = self.q
                self.pv = [
                    pvp.tile([128, NQS], f32, tag=f"pv{j}", name=f"pv_q{q}_{j}")
                    for j in range(NCH)
                ]
                self.acc = accp.tile([128, NQS], f32, tag="acc", name=f"acc_q{q}")
                for g in range(LEAD):
                    self.emit_st_exp_group(g)

            def mloop(self):
                for g in range(NG):
                    self.emit_pv_group(g)
                    if g + LEAD < NG:
                        self.emit_st_exp_group(g + LEAD)

            def epilogue(self):
                q = self.q
                # denominator: reduce acc over partitions with tiny matmuls;
                # output partition = n, which also gives the orientation the
                # per-partition normalize needs. The tile comes from the S^T
                # pool (drained by now) so all 4 psum banks stay with stp.
                dn = stp.tile([128, NQS], f32, tag="st", name=f"dn_q{q}")
                for j in range(NCH):
                    nc.tensor.matmul(
                        dn[:, j : j + 1],
                        self.acc[:, j * 128 : (j + 1) * 128],
                        ones_sb[:],
                        start=True,
                        stop=True,
                    )
                recip = smallp.tile(
                    [128, NCH], f32, tag="recip", name=f"recip_q{q}"
                )
                nc.vector.reciprocal(recip[:], dn[:, :NCH])
                for j in range(NCH):
                    o = outp.tile([128, Cv], f32, tag="o", name=f"o_q{q}_{j}")
                    # split the tail normalize across DVE and ACT so the
                    # final PSUM evacuation halves in wall-clock
                    if j % 2 == 0:
                        nc.vector.tensor_scalar_mul(
                            o[:], self.pv[j][:], recip[:, j : j + 1]
                        )
                    else:
                        nc.scalar.activation(
                            o[:], self.pv[j][:], Copy, scale=recip[:, j : j + 1]
                        )
                    n0 = q * NQS + j * 128
                    nc.sync.dma_start(out_d[n0 : n0 + 128, :], o[:])

        # interleave: emit the next half's prologue before this half's
        # epilogue so PE isn't FIFO-blocked behind the denominator matmuls
        # (which wait on the DVE accumulation tail)
        halves = [Half(q) for q in range(NQ)]
        halves[0].prologue()
        for q in range(NQ):
            halves[q].mloop()
            if q + 1 < NQ:
                halves[q + 1].prologue()
            halves[q].epilogue()

    with tile.TileContext(nc) as tc:
        with (
            tc.tile_pool(name="big", bufs=1) as big,
            tc.tile_pool(name="ptp", bufs=PT_BUFS) as ptp,
            tc.tile_pool(name="accp", bufs=2) as accp,
            tc.tile_pool(name="outp", bufs=6) as outp,
            tc.tile_pool(name="smallp", bufs=4) as smallp,
            tc.tile_pool(name="stp", bufs=4, space="PSUM") as stp,
            tc.tile_pool(name="pvp", bufs=1, space="PSUM") as pvp,
        ):
            if mode == "compute" and loop > 1:
                tiles = emit_dma(nc, tc, big)
                with tc.For_i(0, loop, 1):
                    body(nc, tc, big, ptp, accp, outp, smallp, stp, pvp, tiles)
            elif mode == "dma" and loop > 1:
                with tc.For_i(0, loop, 1):
                    emit_dma(nc, tc, big)
                    for j in range(8):
                        o = outp.tile([128, Cv], f32, tag="o", name=f"o_{j}")
                        nc.vector.memset(o[:], float(j))
                        nc.sync.dma_start(out_d[j * 128 : (j + 1) * 128, :], o[:])
            else:
                loop_ctx = tc.For_i(0, loop, 1) if loop > 1 else None
                with (loop_ctx if loop_ctx is not None else _null()):
                    tiles = emit_dma(nc, tc, big)
                    body(nc, tc, big, ptp, accp, outp, smallp, stp, pvp, tiles)

    nc.finalize()
    return nc


class _null:
    def __enter__(self):
        return None

    def __exit__(self, *a):
        return False


def _get_nc():
    key = ("nc", QK_MODE, LOOP, MODE, STB, LEAD, PT_BUFS, DUMMYPT, NOACC)
    if key not in _CACHE:
        _CACHE[key] = _build_nc(QK_MODE, LOOP, MODE)
    return _CACHE[key]


def _prep_core_inputs(memory_keys, memory_values, query_key, b):
    np_qk_dt = np.float16 if QK_MODE == "f16" else np.float32
    # astype on the transposed view fuses cast+copy in one pass
    qk = query_key[b].reshape(Ck, HW).astype(np_qk_dt)
    mk = memory_keys[:, b].transpose(1, 0, 2, 3).astype(np_qk_dt).reshape(Ck, M)
    mv = (
        memory_values[:, b]
        .transpose(0, 2, 3, 1)
        .astype(ml_dtypes.bfloat16)
        .reshape(M, Cv)
    )
    return {"qk": qk, "mk": mk, "mv": mv}


_RUNNER = {}


def _get_runner():
    """Build the sharded PJRT callable once and reuse it — the generic
    run_bass_kernel_spmd path re-traces jax.jit on every call (~2 s)."""
    if "r" not in _RUNNER:
        import jax
        from jax.sharding import Mesh, PartitionSpec, NamedSharding
        from jax.experimental.shard_map import shard_map

        import concourse.mybir as mybir
        from concourse import bass2jax
        from concourse.bass2jax import _bass_exec_p, install_neuronx_cc_hook

        nc = _get_nc()
        install_neuronx_cc_hook()
        pname = nc.partition_id_tensor.name if nc.partition_id_tensor else None
        in_names, out_names, out_avals = [], [], []
        for alloc in nc.m.functions[0].allocations:
            if not isinstance(alloc, mybir.MemoryLocationSet):
                continue
            name = alloc.memorylocations[0].name
            if alloc.kind == "ExternalInput":
                if name != pname:
                    in_names.append(name)
            elif alloc.kind == "ExternalOutput":
                out_names.append(name)
                out_avals.append(
                    jax.core.ShapedArray(
                        tuple(alloc.tensor_shape), mybir.dt.np(alloc.dtype)
                    )
                )
        n_params = len(in_names)
        all_in = list(in_names) + list(out_names) + ([pname] if pname else [])

        def _body(*args):
            operands = list(args)
            if pname is not None:
                operands.append(bass2jax.partition_id_tensor())
            return tuple(
                _bass_exec_p.bind(
                    *operands,
                    out_avals=tuple(out_avals),
                    in_names=tuple(all_in),
                    out_names=tuple(out_names),
                    lowering_input_output_aliases=(),
                    sim_require_finite=True,
                    sim_require_nnan=True,
                    nc=nc,
                )
            )

        mesh = Mesh(np.asarray(jax.devices()[:N_CORES]), ("core",))
        n_outs = len(out_names)
        sharded = jax.jit(
            shard_map(
                _body,
                mesh=mesh,
                in_specs=(PartitionSpec("core"),) * (n_params + n_outs),
                out_specs=(PartitionSpec("core"),) * n_outs,
                check_rep=False,
            ),
            keep_unused=True,
        )
        sh = NamedSharding(mesh, PartitionSpec("core"))
        zeros = [
            jax.device_put(
                np.zeros((N_CORES * a.shape[0], *a.shape[1:]), a.dtype), sh
            )
            for a in out_avals
        ]
        _RUNNER["r"] = (sharded, sh, in_names, zeros)
    return _RUNNER["r"]


def kernel(memory_keys, memory_values, query_key, query_value):
    global LAST_RESULTS
    memory_keys = np.asarray(memory_keys, dtype=np.float32)
    memory_values = np.asarray(memory_values, dtype=np.float32)
    query_key = np.asarray(query_key, dtype=np.float32)
    query_value = np.asarray(query_value, dtype=np.float32)

    in_maps = [
        _prep_core_inputs(memory_keys, memory_values, query_key, b)
        for b in range(N_CORES)
    ]
    try:
        import jax

        sharded, sh, in_names, zeros = _get_runner()
        dev_in = [
            jax.device_put(
                np.concatenate([in_maps[c][n] for c in range(N_CORES)], 0), sh
            )
            for n in in_names
        ]
        outs = sharded(*dev_in, *zeros)
        out_full = np.asarray(outs[0]).reshape(N_CORES, HW, Cv)
    except Exception:
        # conservative fallback: the generic (slower per call) exec path
        from concourse.bass_utils import run_bass_kernel_spmd

        res = run_bass_kernel_spmd(
            _get_nc(), in_maps, core_ids=list(range(N_CORES))
        )
        LAST_RESULTS = res
        out_full = np.stack([res.results[b]["out"] for b in range(N_CORES)])

    mem = np.stack(
        [out_full[b].T.reshape(Cv, H, W) for b in range(N_CORES)]
    ).astype(np.float32)
    return np.concatenate([query_value, mem], axis=1)


# revision 12
# speedup vs baseline: 1.2088x; 1.1233x over previous
"""Trainium2 Bass kernel for the MemoryModule problem.

Per batch element b (8 of them, one per NeuronCore):
    mk = memory_keys[:, b]  viewed as (Ck=128, M=8192)   [M = T*H*W]
    mv = memory_values[:, b] viewed as (Cv=512, M)
    qk = query_key[b]       viewed as (Ck=128, N=1024)   [N = H*W]
    S  = qk^T @ mk          (N, M)
    P  = softmax(S, axis=-1)
    mem = (P @ mv^T)^T      (Cv, N)
    out[b] = concat([query_value[b], mem], channel axis)

Device dataflow (all transposes done on host / by layout, none on chip):
    - S^T computed directly: S^T tile (128 m, n) = matmul(lhsT=mk_tile, rhs=qk)
      with fp16 inputs.
    - exp on ScalarE (no max subtraction: |S| <~ 70, exp fits fp32/bf16 range),
      written as bf16 P^T tiles.
    - PV: matmul(lhsT=P^T chunk (m,128n), rhs=mv^T tile (m,512c)) in bf16,
      accumulated over the 64 m tiles in PSUM.
    - softmax denominator: VectorE accumulates the P^T tiles into an SBUF
      fp32 accumulator; per half, 4 tiny PE matmuls against a ones vector
      reduce the 128 partition-partials per n column (output partition = n,
      which also transposes the denominator into the orientation the
      normalize step needs).
    - normalize with reciprocal + per-partition scalar multiply, DMA out
      as mem^T (N, Cv); host transposes back.

Key scheduling insight (measured): interleaving single-shot S^T matmuls
with the PV psum-accumulation stream costs ~+43ns on EVERY PE instruction
(pipeline disturbance at psum group/mode switches). Batching the S^T
matmuls in groups of 4 (ST_BATCH) between 16-instr PV phases recovers it
(253 -> 199 ns/instr in microbenches). n is processed in halves of 512 so
PSUM fits: 3 S^T banks + 4 PV accumulators + 1 denominator bank = 8.
"""

import os

import numpy as np
import ml_dtypes

T, B, Ck, Cv, H, W = 8, 8, 128, 512, 32, 32
HW = H * W            # 1024  (n dimension)
M = T * HW            # 8192  (memory / contraction dimension)
MT = M // 128         # 64 m-tiles
NQ = 2                # process n in halves
NQS = HW // NQ        # 512 columns of S^T per half
NCH = NQS // 128      # 4 PV accumulators per half
N_CORES = 8

QK_MODE = os.environ.get("KERNEL_QK_MODE", "f16")
# m-tiles per S^T batch (PE psum-mode-switch amortization)
STB = int(os.environ.get("KERNEL_STB", "4"))
NG = MT // STB        # st/pv groups per half
# groups of lookahead for st/exp emission relative to pv consumption
LEAD = int(os.environ.get("KERNEL_LEAD", "2"))
PT_BUFS = int(os.environ.get("KERNEL_PT_BUFS", "16"))
# >1: repeat the full compute (incl. input DMAs) inside one NEFF via a
# hardware For_i loop, for HW timing via wall-clock deltas.
LOOP = int(os.environ.get("KERNEL_LOOP", "1"))
# timing diagnostics: "full" | "dma" (loop only DMAs) | "compute" (DMAs
# hoisted out of the loop, loop only compute)
MODE = os.environ.get("KERNEL_MODE", "full")
# timing bisection knobs (numerics become wrong; timing-only)
DUMMYPT = os.environ.get("KERNEL_DUMMYPT", "0") == "1"  # pv reads constant tile
NOACC = os.environ.get("KERNEL_NOACC", "0") == "1"      # skip DVE acc chain

_CACHE = {}
LAST_RESULTS = None


def _build_nc(qk_mode, loop=1, mode="full"):
    import concourse.tile as tile
    import concourse.mybir as mybir
    from concourse import bacc

    f32 = mybir.dt.float32
    bf16 = mybir.dt.bfloat16
    f16 = mybir.dt.float16
    qk_dt = {"f16": f16, "f32r": f32, "f32": f32}[qk_mode]

    nc = bacc.Bacc()

    qk_d = nc.dram_tensor("qk", [Ck, HW], qk_dt, kind="ExternalInput")
    mk_d = nc.dram_tensor("mk", [Ck, M], qk_dt, kind="ExternalInput")
    mv_d = nc.dram_tensor("mv", [M, Cv], bf16, kind="ExternalInput")
    out_d = nc.dram_tensor("out", [HW, Cv], f32, kind="ExternalOutput")

    mv_tiled = mv_d.rearrange("(mt p) c -> mt p c", p=128)  # (64, 128, 512)

    Exp = mybir.ActivationFunctionType.Exp
    Copy = mybir.ActivationFunctionType.Copy
    AluOp = mybir.AluOpType

    def emit_dma(nc, tc, big):
        qk_sb = big.tile([Ck, HW], qk_dt, tag="qk_sb", name="qk_sb")
        nc.sync.dma_start(qk_sb[:], qk_d[:])
        mk_sb = big.tile([Ck, M], qk_dt, tag="mk_sb", name="mk_sb")
        # split so the first S^T matmuls don't wait for the whole tensor
        for i in range(8):
            nc.sync.dma_start(
                mk_sb[:, i * HW : (i + 1) * HW], mk_d[:, i * HW : (i + 1) * HW]
            )
        mv_sb = big.tile([128, MT, Cv], bf16, tag="mv_sb", name="mv_sb")
        for m in range(MT):
            nc.sync.dma_start(mv_sb[:, m], mv_tiled[m])
        return qk_sb, mk_sb, mv_sb

    def body(nc, tc, big, ptp, accp, outp, smallp, stp, pvp, tiles):
        qk_sb, mk_sb, mv_sb = tiles
        ones_sb = big.tile([128, 1], f32, tag="ones_sb", name="ones_sb")
        nc.vector.memset(ones_sb[:], 1.0)
        dummy_pt = None
        if DUMMYPT:
            dummy_pt = big.tile([128, NQS], bf16, tag="dummy_pt", name="dummy_pt")
            nc.vector.memset(dummy_pt[:], 0.001)

        def mm_cast(ap):
            if qk_mode == "f32r":
                return ap.bitcast(mybir.dt.float32r)
            return ap

        class Half:
            def __init__(self, q):
                self.q = q
                self.sts = {}
                self.pts = {}

            def emit_st_exp_group(self, g):
                q = self.q
                ms = range(g * STB, (g + 1) * STB)
                # batch all st matmuls, then all exps, to keep the PE's
                # psum-mode switches at group granularity
                for m in ms:
                    st = stp.tile(
                        [128, NQS], f32, tag="st", name=f"st_q{q}_m{m}"
                    )
                    nc.tensor.matmul(
                        st[:],
                        mm_cast(mk_sb[:, m * 128 : (m + 1) * 128]),
                        mm_cast(qk_sb[:, q * NQS : (q + 1) * NQS]),
                        start=True,
                        stop=True,
                    )
                    self.sts[m] = st
                for m in ms:
                    pt = ptp.tile(
                        [128, NQS], bf16, tag="pt", name=f"pt_q{q}_m{m}"
                    )
                    nc.scalar.activation(pt[:], self.sts.pop(m)[:], Exp)
                    self.pts[m] = pt

            def emit_acc(self, m):
                # VectorE: accumulate exp tiles for the softmax denominator
                if NOACC:
                    if m == 0:
                        nc.vector.memset(self.acc[:], 1.0)
                    return
                if m == 0:
                    nc.vector.tensor_copy(self.acc[:], self.pts[m][:])
                else:
                    nc.vector.tensor_tensor(
                        self.acc[:], self.acc[:], self.pts[m][:], AluOp.add
                    )

            def emit_pv_group(self, g):
                for m in range(g * STB, (g + 1) * STB):
                    ptm = dummy_pt if DUMMYPT else self.pts[m]
                    for j in range(NCH):
                        nc.tensor.matmul(
                            self.pv[j][:],
                            ptm[:, j * 128 : (j + 1) * 128],
                            mv_sb[:, m],
                            start=(m == 0),
                            stop=(m == MT - 1),
                        )
                    self.emit_acc(m)
                    del self.pts[m]

            def prologue(self):
                q = self.q
                self.pv = [
                    pvp.tile([128, NQS], f32, tag=f"pv{j}", name=f"pv_q{q}_{j}")
                    for j in range(NCH)
                ]
                self.acc = accp.tile([128, NQS], f32, tag="acc", name=f"acc_q{q}")
                for g in range(LEAD):
                    self.emit_st_exp_group(g)

            def mloop(self):
                for g in range(NG):
                    self.emit_pv_group(g)
                    if g + LEAD < NG:
                        self.emit_st_exp_group(g + LEAD)

            def epilogue(self):
                q = self.q
                # denominator: reduce acc over partitions with tiny matmuls;
                # output partition = n, which also gives the orientation the
                # per-partition normalize needs. The tile comes from the S^T
                # pool (drained by now) so all 4 psum banks stay with stp.
                dn = stp.tile([128, NQS], f32, tag="st", name=f"dn_q{q}")
                for j in range(NCH):
                    nc.tensor.matmul(
                        dn[:, j : j + 1],
                        self.acc[:, j * 128 : (j + 1) * 128],
                        ones_sb[:],
                        start=True,
                        stop=True,
                    )
                recip = smallp.tile(
                    [128, NCH], f32, tag="recip", name=f"recip_q{q}"
                )
                nc.vector.reciprocal(recip[:], dn[:, :NCH])
                for j in range(NCH):
                    o = outp.tile([128, Cv], f32, tag="o", name=f"o_q{q}_{j}")
                    # split the tail normalize across DVE and ACT so the
                    # final PSUM evacuation halves in wall-clock
                    if j % 2 == 0:
                        nc.vector.tensor_scalar_mul(
                            o[:], self.pv[j][:], recip[:, j : j + 1]
                        )
                    else:
                        nc.scalar.activation(
                            o[:], self.pv[j][:], Copy, scale=recip[:, j : j + 1]
                        )
                    n0 = q * NQS + j * 128
                    nc.sync.dma_start(out_d[n0 : n0 + 128, :], o[:])

        # interleave: emit the next half's prologue before this half's
        # epilogue so PE isn't FIFO-blocked behind the denominator matmuls
        # (which wait on the DVE accumulation tail)
        halves = [Half(q) for q in range(NQ)]
        halves[0].prologue()
        for q in range(NQ):
            halves[q].mloop()
            if q + 1 < NQ:
                halves[q + 1].prologue()
            halves[q].epilogue()

    with tile.TileContext(nc) as tc:
        with (
            tc.tile_pool(name="big", bufs=1) as big,
            tc.tile_pool(name="ptp", bufs=PT_BUFS) as ptp,
            tc.tile_pool(name="accp", bufs=2) as accp,
            tc.tile_pool(name="outp", bufs=6) as outp,
            tc.tile_pool(name="smallp", bufs=4) as smallp,
            tc.tile_pool(name="stp", bufs=4, space="PSUM") as stp,
            tc.tile_pool(name="pvp", bufs=1, space="PSUM") as pvp,
        ):
            if mode == "compute" and loop > 1:
                tiles = emit_dma(nc, tc, big)
                with tc.For_i(0, loop, 1):
                    body(nc, tc, big, ptp, accp, outp, smallp, stp, pvp, tiles)
            elif mode == "dma" and loop > 1:
                with tc.For_i(0, loop, 1):
                    emit_dma(nc, tc, big)
                    for j in range(8):
                        o = outp.tile([128, Cv], f32, tag="o", name=f"o_{j}")
                        nc.vector.memset(o[:], float(j))
                        nc.sync.dma_start(out_d[j * 128 : (j + 1) * 128, :], o[:])
            else:
                loop_ctx = tc.For_i(0, loop, 1) if loop > 1 else None
                with (loop_ctx if loop_ctx is not None else _null()):
                    tiles = emit_dma(nc, tc, big)
                    body(nc, tc, big, ptp, accp, outp, smallp, stp, pvp, tiles)

    nc.finalize()
    return nc


class _null:
    def __enter__(self):
        return None

    def __exit__(self, *a):
        return False


def _get_nc():
    key = ("nc", QK_MODE, LOOP, MODE, STB, LEAD, PT_BUFS, DUMMYPT, NOACC)
    if key not in _CACHE:
        _CACHE[key] = _build_nc(QK_MODE, LOOP, MODE)
    return _CACHE[key]


def _prep_core_inputs(memory_keys, memory_values, query_key, b):
    np_qk_dt = np.float16 if QK_MODE == "f16" else np.float32
    # astype on the transposed view fuses cast+copy in one pass
    qk = query_key[b].reshape(Ck, HW).astype(np_qk_dt)
    mk = memory_keys[:, b].transpose(1, 0, 2, 3).astype(np_qk_dt).reshape(Ck, M)
    mv = (
        memory_values[:, b]
        .transpose(0, 2, 3, 1)
        .astype(ml_dtypes.bfloat16)
        .reshape(M, Cv)
    )
    return {"qk": qk, "mk": mk, "mv": mv}


_RUNNER = {}


def _get_runner():
    """Build the sharded PJRT callable once and reuse it — the generic
    run_bass_kernel_spmd path re-traces jax.jit on every call (~2 s)."""
    if "r" not in _RUNNER:
        import jax
        from jax.sharding import Mesh, PartitionSpec, NamedSharding
        from jax.experimental.shard_map import shard_map

        import concourse.mybir as mybir
        from concourse import bass2jax
        from concourse.bass2jax import _bass_exec_p, install_neuronx_cc_hook

        nc = _get_nc()
        install_neuronx_cc_hook()
        pname = nc.partition_id_tensor.name if nc.partition_id_tensor else None
        in_names, out_names, out_avals = [], [], []
        for alloc in nc.m.functions[0].allocations:
            if not isinstance(alloc, mybir.MemoryLocationSet):
                continue
            name = alloc.memorylocations[0].name
            if alloc.kind == "ExternalInput":
                if name != pname:
                    in_names.append(name)
            elif alloc.kind == "ExternalOutput":
                out_names.append(name)
                out_avals.append(
                    jax.core.ShapedArray(
                        tuple(alloc.tensor_shape), mybir.dt.np(alloc.dtype)
                    )
                )
        n_params = len(in_names)
        all_in = list(in_names) + list(out_names) + ([pname] if pname else [])

        def _body(*args):
            operands = list(args)
            if pname is not None:
                operands.append(bass2jax.partition_id_tensor())
            return tuple(
                _bass_exec_p.bind(
                    *operands,
                    out_avals=tuple(out_avals),
                    in_names=tuple(all_in),
                    out_names=tuple(out_names),
                    lowering_input_output_aliases=(),
                    sim_require_finite=True,
                    sim_require_nnan=True,
                    nc=nc,
                )
            )

        mesh = Mesh(np.asarray(jax.devices()[:N_CORES]), ("core",))
        n_outs = len(out_names)
        sharded = jax.jit(
            shard_map(
                _body,
                mesh=mesh,
                in_specs=(PartitionSpec("core"),) * (n_params + n_outs),
                out_specs=(PartitionSpec("core"),) * n_outs,
                check_rep=False,
            ),
            keep_unused=True,
        )
        sh = NamedSharding(mesh, PartitionSpec("core"))
        zeros = [
            jax.device_put(
                np.zeros((N_CORES * a.shape[0], *a.shape[1:]), a.dtype), sh
            )
            for a in out_avals
        ]
        _RUNNER["r"] = (sharded, sh, in_names, zeros)
    return _RUNNER["r"]


def kernel(memory_keys, memory_values, query_key, query_value):
    global LAST_RESULTS
    memory_keys = np.asarray(memory_keys, dtype=np.float32)
    memory_values = np.asarray(memory_values, dtype=np.float32)
    query_key = np.asarray(query_key, dtype=np.float32)
    query_value = np.asarray(query_value, dtype=np.float32)

    in_maps = [
        _prep_core_inputs(memory_keys, memory_values, query_key, b)
        for b in range(N_CORES)
    ]
    try:
        import jax

        sharded, sh, in_names, zeros = _get_runner()
        dev_in = [
            jax.device_put(
                np.concatenate([in_maps[c][n] for c in range(N_CORES)], 0), sh
            )
            for n in in_names
        ]
        outs = sharded(*dev_in, *zeros)
        out_full = np.asarray(outs[0]).reshape(N_CORES, HW, Cv)
    except Exception:
        # conservative fallback: the generic (slower per call) exec path
        from concourse.bass_utils import run_bass_kernel_spmd

        res = run_bass_kernel_spmd(
            _get_nc(), in_maps, core_ids=list(range(N_CORES))
        )
        LAST_RESULTS = res
        out_full = np.stack([res.results[b]["out"] for b in range(N_CORES)])

    mem = np.stack(
        [out_full[b].T.reshape(Cv, H, W) for b in range(N_CORES)]
    ).astype(np.float32)
    return np.concatenate([query_value, mem], axis=1)


# revision 13
# speedup vs baseline: 1.2903x; 1.0674x over previous
"""Trainium2 Bass kernel for the MemoryModule problem.

Per batch element b (8 of them, one per NeuronCore):
    mk = memory_keys[:, b]  viewed as (Ck=128, M=8192)   [M = T*H*W]
    mv = memory_values[:, b] viewed as (Cv=512, M)
    qk = query_key[b]       viewed as (Ck=128, N=1024)   [N = H*W]
    S  = qk^T @ mk          (N, M)
    P  = softmax(S, axis=-1)
    mem = (P @ mv^T)^T      (Cv, N)
    out[b] = concat([query_value[b], mem], channel axis)

Device dataflow (all transposes done on host / by layout, none on chip):
    - S^T computed directly: S^T tile (128 m, n) = matmul(lhsT=mk_tile, rhs=qk)
      with fp16 inputs.
    - exp on ScalarE (no max subtraction: |S| <~ 70, exp fits fp32/bf16 range),
      written as bf16 P^T tiles.
    - PV: matmul(lhsT=P^T chunk (m,128n), rhs=mv^T tile (m,512c)) in bf16,
      accumulated over the 64 m tiles in PSUM.
    - softmax denominator: VectorE accumulates the P^T tiles into an SBUF
      fp32 accumulator; per half, 4 tiny PE matmuls against a ones vector
      reduce the 128 partition-partials per n column (output partition = n,
      which also transposes the denominator into the orientation the
      normalize step needs).
    - normalize with reciprocal + per-partition scalar multiply, DMA out
      as mem^T (N, Cv); host transposes back.

Key scheduling insight (measured): interleaving single-shot S^T matmuls
with the PV psum-accumulation stream costs ~+43ns on EVERY PE instruction
(pipeline disturbance at psum group/mode switches). Batching the S^T
matmuls in groups of 4 (ST_BATCH) between 16-instr PV phases recovers it
(253 -> 199 ns/instr in microbenches). n is processed in halves of 512 so
PSUM fits: 3 S^T banks + 4 PV accumulators + 1 denominator bank = 8.
"""

import os

import numpy as np
import ml_dtypes

T, B, Ck, Cv, H, W = 8, 8, 128, 512, 32, 32
HW = H * W            # 1024  (n dimension)
M = T * HW            # 8192  (memory / contraction dimension)
MT = M // 128         # 64 m-tiles
NQ = 2                # process n in halves
NQS = HW // NQ        # 512 columns of S^T per half
NCH = NQS // 128      # 4 PV accumulators per half
N_CORES = 8

QK_MODE = os.environ.get("KERNEL_QK_MODE", "f16")
# m-tiles per S^T batch (PE psum-mode-switch amortization)
STB = int(os.environ.get("KERNEL_STB", "4"))
NG = MT // STB        # st/pv groups per half
# groups of lookahead for st/exp emission relative to pv consumption
LEAD = int(os.environ.get("KERNEL_LEAD", "2"))
PT_BUFS = int(os.environ.get("KERNEL_PT_BUFS", "16"))
# >1: repeat the full compute (incl. input DMAs) inside one NEFF via a
# hardware For_i loop, for HW timing via wall-clock deltas.
LOOP = int(os.environ.get("KERNEL_LOOP", "1"))
# timing diagnostics: "full" | "dma" (loop only DMAs) | "compute" (DMAs
# hoisted out of the loop, loop only compute)
MODE = os.environ.get("KERNEL_MODE", "full")
# timing bisection knobs (numerics become wrong; timing-only)
DUMMYPT = os.environ.get("KERNEL_DUMMYPT", "0") == "1"  # pv reads constant tile
NOACC = os.environ.get("KERNEL_NOACC", "0") == "1"      # skip DVE acc chain

_CACHE = {}
LAST_RESULTS = None


def _build_nc(qk_mode, loop=1, mode="full"):
    import concourse.tile as tile
    import concourse.mybir as mybir
    from concourse import bacc

    f32 = mybir.dt.float32
    bf16 = mybir.dt.bfloat16
    f16 = mybir.dt.float16
    qk_dt = {"f16": f16, "f32r": f32, "f32": f32}[qk_mode]

    nc = bacc.Bacc()

    qk_d = nc.dram_tensor("qk", [Ck, HW], qk_dt, kind="ExternalInput")
    mk_d = nc.dram_tensor("mk", [Ck, M], qk_dt, kind="ExternalInput")
    mv_d = nc.dram_tensor("mv", [M, Cv], bf16, kind="ExternalInput")
    out_d = nc.dram_tensor("out", [HW, Cv], f32, kind="ExternalOutput")

    mv_tiled = mv_d.rearrange("(mt p) c -> mt p c", p=128)  # (64, 128, 512)

    Exp = mybir.ActivationFunctionType.Exp
    Copy = mybir.ActivationFunctionType.Copy
    AluOp = mybir.AluOpType

    def emit_dma(nc, tc, big):
        qk_sb = big.tile([Ck, HW], qk_dt, tag="qk_sb", name="qk_sb")
        nc.sync.dma_start(qk_sb[:], qk_d[:])
        mk_sb = big.tile([Ck, M], qk_dt, tag="mk_sb", name="mk_sb")
        # split so the first S^T matmuls don't wait for the whole tensor
        for i in range(8):
            nc.sync.dma_start(
                mk_sb[:, i * HW : (i + 1) * HW], mk_d[:, i * HW : (i + 1) * HW]
            )
        mv_sb = big.tile([128, MT, Cv], bf16, tag="mv_sb", name="mv_sb")
        for m in range(MT):
            nc.sync.dma_start(mv_sb[:, m], mv_tiled[m])
        return qk_sb, mk_sb, mv_sb

    def body(nc, tc, big, ptp, accp, outp, smallp, stp, pvp, tiles):
        qk_sb, mk_sb, mv_sb = tiles
        ones_sb = big.tile([128, 1], bf16, tag="ones_sb", name="ones_sb")
        nc.vector.memset(ones_sb[:], 1.0)
        dummy_pt = None
        if DUMMYPT:
            dummy_pt = big.tile([128, NQS], bf16, tag="dummy_pt", name="dummy_pt")
            nc.vector.memset(dummy_pt[:], 0.001)

        def mm_cast(ap):
            if qk_mode == "f32r":
                return ap.bitcast(mybir.dt.float32r)
            return ap

        class Half:
            def __init__(self, q):
                self.q = q
                self.sts = {}
                self.pts = {}

            def emit_st_exp_group(self, g):
                q = self.q
                ms = range(g * STB, (g + 1) * STB)
                # batch all st matmuls, then all exps, to keep the PE's
                # psum-mode switches at group granularity
                for m in ms:
                    st = stp.tile(
                        [128, NQS], f32, tag="st", name=f"st_q{q}_m{m}"
                    )
                    nc.tensor.matmul(
                        st[:],
                        mm_cast(mk_sb[:, m * 128 : (m + 1) * 128]),
                        mm_cast(qk_sb[:, q * NQS : (q + 1) * NQS]),
                        start=True,
                        stop=True,
                    )
                    self.sts[m] = st
                for m in ms:
                    pt = ptp.tile(
                        [128, NQS], bf16, tag="pt", name=f"pt_q{q}_m{m}"
                    )
                    nc.scalar.activation(pt[:], self.sts.pop(m)[:], Exp)
                    self.pts[m] = pt

            def emit_acc(self, m):
                # VectorE: accumulate exp tiles for the softmax denominator
                if NOACC:
                    if m == 0:
                        nc.vector.memset(self.acc[:], 1.0)
                    return
                if m == 0:
                    nc.vector.tensor_copy(self.acc[:], self.pts[m][:])
                else:
                    nc.vector.tensor_tensor(
                        self.acc[:], self.acc[:], self.pts[m][:], AluOp.add
                    )

            def emit_pv_group(self, g):
                for m in range(g * STB, (g + 1) * STB):
                    ptm = dummy_pt if DUMMYPT else self.pts[m]
                    for j in range(NCH):
                        nc.tensor.matmul(
                            self.pv[j][:],
                            ptm[:, j * 128 : (j + 1) * 128],
                            mv_sb[:, m],
                            start=(m == 0),
                            stop=(m == MT - 1),
                        )
                    self.emit_acc(m)
                    del self.pts[m]

            def prologue(self):
                q = self.q
                self.pv = [
                    pvp.tile([128, NQS], f32, tag=f"pv{j}", name=f"pv_q{q}_{j}")
                    for j in range(NCH)
                ]
                self.acc = accp.tile([128, NQS], f32, tag="acc", name=f"acc_q{q}")
                for g in range(LEAD):
                    self.emit_st_exp_group(g)

            def mloop(self):
                for g in range(NG):
                    self.emit_pv_group(g)
                    if g + LEAD < NG:
                        self.emit_st_exp_group(g + LEAD)

            def epilogue(self):
                q = self.q
                # denominator: reduce acc over partitions with tiny matmuls;
                # output partition = n, which also gives the orientation the
                # per-partition normalize needs. The tile comes from the S^T
                # pool (drained by now) so all 4 psum banks stay with stp.
                dn = stp.tile([128, NQS], f32, tag="st", name=f"dn_q{q}")
                # cast the accumulator to bf16 so the partition-reduce matmuls
                # stay in the fast 1-cyc/row PE mode (fp32 lhsT would force
                # 4-cyc/row excursions mid-stream); den error ~0.4%/sqrt(128),
                # negligible
                accb = outp.tile([128, NQS], bf16, tag="accb", name=f"accb_q{q}")
                nc.vector.tensor_copy(accb[:], self.acc[:])
                for j in range(NCH):
                    nc.tensor.matmul(
                        dn[:, j : j + 1],
                        accb[:, j * 128 : (j + 1) * 128],
                        ones_sb[:],
                        start=True,
                        stop=True,
                    )
                recip = smallp.tile(
                    [128, NCH], f32, tag="recip", name=f"recip_q{q}"
                )
                nc.vector.reciprocal(recip[:], dn[:, :NCH])
                for j in range(NCH):
                    o = outp.tile([128, Cv], f32, tag="o", name=f"o_q{q}_{j}")
                    # split the tail normalize across DVE and ACT so the
                    # final PSUM evacuation halves in wall-clock
                    if j % 2 == 0:
                        nc.vector.tensor_scalar_mul(
                            o[:], self.pv[j][:], recip[:, j : j + 1]
                        )
                    else:
                        nc.scalar.activation(
                            o[:], self.pv[j][:], Copy, scale=recip[:, j : j + 1]
                        )
                    n0 = q * NQS + j * 128
                    nc.scalar.dma_start(out_d[n0 : n0 + 128, :], o[:])

        # interleave: emit the next half's prologue before this half's
        # epilogue so PE isn't FIFO-blocked behind the denominator matmuls
        # (which wait on the DVE accumulation tail)
        halves = [Half(q) for q in range(NQ)]
        halves[0].prologue()
        for q in range(NQ):
            halves[q].mloop()
            if q + 1 < NQ:
                halves[q + 1].prologue()
            halves[q].epilogue()

    with tile.TileContext(nc) as tc:
        with (
            tc.tile_pool(name="big", bufs=1) as big,
            tc.tile_pool(name="ptp", bufs=PT_BUFS) as ptp,
            tc.tile_pool(name="accp", bufs=2) as accp,
            tc.tile_pool(name="outp", bufs=6) as outp,
            tc.tile_pool(name="smallp", bufs=4) as smallp,
            tc.tile_pool(name="stp", bufs=4, space="PSUM") as stp,
            tc.tile_pool(name="pvp", bufs=1, space="PSUM") as pvp,
        ):
            if mode == "compute" and loop > 1:
                tiles = emit_dma(nc, tc, big)
                with tc.For_i(0, loop, 1):
                    body(nc, tc, big, ptp, accp, outp, smallp, stp, pvp, tiles)
            elif mode == "dma" and loop > 1:
                with tc.For_i(0, loop, 1):
                    emit_dma(nc, tc, big)
                    for j in range(8):
                        o = outp.tile([128, Cv], f32, tag="o", name=f"o_{j}")
                        nc.vector.memset(o[:], float(j))
                        nc.sync.dma_start(out_d[j * 128 : (j + 1) * 128, :], o[:])
            else:
                loop_ctx = tc.For_i(0, loop, 1) if loop > 1 else None
                with (loop_ctx if loop_ctx is not None else _null()):
                    tiles = emit_dma(nc, tc, big)
                    body(nc, tc, big, ptp, accp, outp, smallp, stp, pvp, tiles)

    nc.finalize()
    return nc


class _null:
    def __enter__(self):
        return None

    def __exit__(self, *a):
        return False


def _get_nc():
    key = ("nc", QK_MODE, LOOP, MODE, STB, LEAD, PT_BUFS, DUMMYPT, NOACC)
    if key not in _CACHE:
        _CACHE[key] = _build_nc(QK_MODE, LOOP, MODE)
    return _CACHE[key]


def _prep_core_inputs(memory_keys, memory_values, query_key, b):
    np_qk_dt = np.float16 if QK_MODE == "f16" else np.float32
    # astype on the transposed view fuses cast+copy in one pass
    qk = query_key[b].reshape(Ck, HW).astype(np_qk_dt)
    mk = memory_keys[:, b].transpose(1, 0, 2, 3).astype(np_qk_dt).reshape(Ck, M)
    mv = (
        memory_values[:, b]
        .transpose(0, 2, 3, 1)
        .astype(ml_dtypes.bfloat16)
        .reshape(M, Cv)
    )
    return {"qk": qk, "mk": mk, "mv": mv}


_RUNNER = {}


def _get_runner():
    """Build the sharded PJRT callable once and reuse it — the generic
    run_bass_kernel_spmd path re-traces jax.jit on every call (~2 s)."""
    if "r" not in _RUNNER:
        import jax
        from jax.sharding import Mesh, PartitionSpec, NamedSharding
        from jax.experimental.shard_map import shard_map

        import concourse.mybir as mybir
        from concourse import bass2jax
        from concourse.bass2jax import _bass_exec_p, install_neuronx_cc_hook

        nc = _get_nc()
        install_neuronx_cc_hook()
        pname = nc.partition_id_tensor.name if nc.partition_id_tensor else None
        in_names, out_names, out_avals = [], [], []
        for alloc in nc.m.functions[0].allocations:
            if not isinstance(alloc, mybir.MemoryLocationSet):
                continue
            name = alloc.memorylocations[0].name
            if alloc.kind == "ExternalInput":
                if name != pname:
                    in_names.append(name)
            elif alloc.kind == "ExternalOutput":
                out_names.append(name)
                out_avals.append(
                    jax.core.ShapedArray(
                        tuple(alloc.tensor_shape), mybir.dt.np(alloc.dtype)
                    )
                )
        n_params = len(in_names)
        all_in = list(in_names) + list(out_names) + ([pname] if pname else [])

        def _body(*args):
            operands = list(args)
            if pname is not None:
                operands.append(bass2jax.partition_id_tensor())
            return tuple(
                _bass_exec_p.bind(
                    *operands,
                    out_avals=tuple(out_avals),
                    in_names=tuple(all_in),
                    out_names=tuple(out_names),
                    lowering_input_output_aliases=(),
                    sim_require_finite=True,
                    sim_require_nnan=True,
                    nc=nc,
                )
            )

        mesh = Mesh(np.asarray(jax.devices()[:N_CORES]), ("core",))
        n_outs = len(out_names)
        sharded = jax.jit(
            shard_map(
                _body,
                mesh=mesh,
                in_specs=(PartitionSpec("core"),) * (n_params + n_outs),
                out_specs=(PartitionSpec("core"),) * n_outs,
                check_rep=False,
            ),
            keep_unused=True,
        )
        sh = NamedSharding(mesh, PartitionSpec("core"))
        zeros = [
            jax.device_put(
                np.zeros((N_CORES * a.shape[0], *a.shape[1:]), a.dtype), sh
            )
            for a in out_avals
        ]
        _RUNNER["r"] = (sharded, sh, in_names, zeros)
    return _RUNNER["r"]


def kernel(memory_keys, memory_values, query_key, query_value):
    global LAST_RESULTS
    memory_keys = np.asarray(memory_keys, dtype=np.float32)
    memory_values = np.asarray(memory_values, dtype=np.float32)
    query_key = np.asarray(query_key, dtype=np.float32)
    query_value = np.asarray(query_value, dtype=np.float32)

    in_maps = [
        _prep_core_inputs(memory_keys, memory_values, query_key, b)
        for b in range(N_CORES)
    ]
    try:
        import jax

        sharded, sh, in_names, zeros = _get_runner()
        dev_in = [
            jax.device_put(
                np.concatenate([in_maps[c][n] for c in range(N_CORES)], 0), sh
            )
            for n in in_names
        ]
        outs = sharded(*dev_in, *zeros)
        out_full = np.asarray(outs[0]).reshape(N_CORES, HW, Cv)
    except Exception:
        # conservative fallback: the generic (slower per call) exec path
        from concourse.bass_utils import run_bass_kernel_spmd

        res = run_bass_kernel_spmd(
            _get_nc(), in_maps, core_ids=list(range(N_CORES))
        )
        LAST_RESULTS = res
        out_full = np.stack([res.results[b]["out"] for b in range(N_CORES)])

    mem = np.stack(
        [out_full[b].T.reshape(Cv, H, W) for b in range(N_CORES)]
    ).astype(np.float32)
    return np.concatenate([query_value, mem], axis=1)


# revision 14
# speedup vs baseline: 1.2918x; 1.0011x over previous
"""Trainium2 Bass kernel for the MemoryModule problem.

Per batch element b (8 of them, one per NeuronCore):
    mk = memory_keys[:, b]  viewed as (Ck=128, M=8192)   [M = T*H*W]
    mv = memory_values[:, b] viewed as (Cv=512, M)
    qk = query_key[b]       viewed as (Ck=128, N=1024)   [N = H*W]
    S  = qk^T @ mk          (N, M)
    P  = softmax(S, axis=-1)
    mem = (P @ mv^T)^T      (Cv, N)
    out[b] = concat([query_value[b], mem], channel axis)

Device dataflow (all transposes done on host / by layout, none on chip):
    - S^T computed directly: S^T tile (128 m, n) = matmul(lhsT=mk_tile, rhs=qk)
      with fp16 inputs.
    - exp on ScalarE (no max subtraction: |S| <~ 70, exp fits fp32/bf16 range),
      written as bf16 P^T tiles.
    - PV: matmul(lhsT=P^T chunk (m,128n), rhs=mv^T tile (m,512c)) in bf16,
      accumulated over the 64 m tiles in PSUM.
    - softmax denominator: VectorE accumulates the P^T tiles into an SBUF
      fp32 accumulator; per half, 4 tiny PE matmuls against a ones vector
      reduce the 128 partition-partials per n column (output partition = n,
      which also transposes the denominator into the orientation the
      normalize step needs).
    - normalize with reciprocal + per-partition scalar multiply, DMA out
      as mem^T (N, Cv); host transposes back.

Key scheduling insight (measured): interleaving single-shot S^T matmuls
with the PV psum-accumulation stream costs ~+43ns on EVERY PE instruction
(pipeline disturbance at psum group/mode switches). Batching the S^T
matmuls in groups of 4 (ST_BATCH) between 16-instr PV phases recovers it
(253 -> 199 ns/instr in microbenches). n is processed in halves of 512 so
PSUM fits: 3 S^T banks + 4 PV accumulators + 1 denominator bank = 8.
"""

import os

import numpy as np
import ml_dtypes

T, B, Ck, Cv, H, W = 8, 8, 128, 512, 32, 32
HW = H * W            # 1024  (n dimension)
M = T * HW            # 8192  (memory / contraction dimension)
MT = M // 128         # 64 m-tiles
NQ = 2                # process n in halves
NQS = HW // NQ        # 512 columns of S^T per half
NCH = NQS // 128      # 4 PV accumulators per half
N_CORES = 8

QK_MODE = os.environ.get("KERNEL_QK_MODE", "f16")
# m-tiles per S^T batch (PE psum-mode-switch amortization)
STB = int(os.environ.get("KERNEL_STB", "4"))
NG = MT // STB        # st/pv groups per half
# groups of lookahead for st/exp emission relative to pv consumption
LEAD = int(os.environ.get("KERNEL_LEAD", "2"))
PT_BUFS = int(os.environ.get("KERNEL_PT_BUFS", "16"))
# >1: repeat the full compute (incl. input DMAs) inside one NEFF via a
# hardware For_i loop, for HW timing via wall-clock deltas.
LOOP = int(os.environ.get("KERNEL_LOOP", "1"))
# timing diagnostics: "full" | "dma" (loop only DMAs) | "compute" (DMAs
# hoisted out of the loop, loop only compute)
MODE = os.environ.get("KERNEL_MODE", "full")
# timing bisection knobs (numerics become wrong; timing-only)
DUMMYPT = os.environ.get("KERNEL_DUMMYPT", "0") == "1"  # pv reads constant tile
NOACC = os.environ.get("KERNEL_NOACC", "0") == "1"      # skip DVE acc chain

_CACHE = {}
LAST_RESULTS = None


def _build_nc(qk_mode, loop=1, mode="full"):
    import concourse.tile as tile
    import concourse.mybir as mybir
    from concourse import bacc

    f32 = mybir.dt.float32
    bf16 = mybir.dt.bfloat16
    f16 = mybir.dt.float16
    qk_dt = {"f16": f16, "f32r": f32, "f32": f32}[qk_mode]

    nc = bacc.Bacc()

    qk_d = nc.dram_tensor("qk", [Ck, HW], qk_dt, kind="ExternalInput")
    mk_d = nc.dram_tensor("mk", [Ck, M], qk_dt, kind="ExternalInput")
    mv_d = nc.dram_tensor("mv", [M, Cv], bf16, kind="ExternalInput")
    out_d = nc.dram_tensor("out", [HW, Cv], bf16, kind="ExternalOutput")

    mv_tiled = mv_d.rearrange("(mt p) c -> mt p c", p=128)  # (64, 128, 512)

    Exp = mybir.ActivationFunctionType.Exp
    Copy = mybir.ActivationFunctionType.Copy
    AluOp = mybir.AluOpType

    def emit_dma(nc, tc, big, inp):
        # qk/mk rotate between two buffers so iteration i+1's loads don't
        # WAR-wait on iteration i's last S^T matmuls at the SP queue head
        qk_sb = inp.tile([Ck, HW], qk_dt, tag="qk_sb", name="qk_sb")
        nc.sync.dma_start(qk_sb[:], qk_d[:])
        mk_sb = inp.tile([Ck, M], qk_dt, tag="mk_sb", name="mk_sb")
        # split so the first S^T matmuls don't wait for the whole tensor
        for i in range(8):
            nc.sync.dma_start(
                mk_sb[:, i * HW : (i + 1) * HW], mk_d[:, i * HW : (i + 1) * HW]
            )
        mv_sb = big.tile([128, MT, Cv], bf16, tag="mv_sb", name="mv_sb")
        for m in range(MT):
            nc.sync.dma_start(mv_sb[:, m], mv_tiled[m])
        return qk_sb, mk_sb, mv_sb

    def body(nc, tc, big, ptp, accp, outp, smallp, stp, pvp, tiles):
        qk_sb, mk_sb, mv_sb = tiles
        ones_sb = big.tile([128, 1], bf16, tag="ones_sb", name="ones_sb")
        nc.vector.memset(ones_sb[:], 1.0)
        dummy_pt = None
        if DUMMYPT:
            dummy_pt = big.tile([128, NQS], bf16, tag="dummy_pt", name="dummy_pt")
            nc.vector.memset(dummy_pt[:], 0.001)

        def mm_cast(ap):
            if qk_mode == "f32r":
                return ap.bitcast(mybir.dt.float32r)
            return ap

        class Half:
            def __init__(self, q):
                self.q = q
                self.sts = {}
                self.pts = {}

            def emit_st_exp_group(self, g):
                q = self.q
                ms = range(g * STB, (g + 1) * STB)
                # batch all st matmuls, then all exps, to keep the PE's
                # psum-mode switches at group granularity
                for m in ms:
                    st = stp.tile(
                        [128, NQS], f32, tag="st", name=f"st_q{q}_m{m}"
                    )
                    nc.tensor.matmul(
                        st[:],
                        mm_cast(mk_sb[:, m * 128 : (m + 1) * 128]),
                        mm_cast(qk_sb[:, q * NQS : (q + 1) * NQS]),
                        start=True,
                        stop=True,
                    )
                    self.sts[m] = st
                for m in ms:
                    pt = ptp.tile(
                        [128, NQS], bf16, tag="pt", name=f"pt_q{q}_m{m}"
                    )
                    nc.scalar.activation(pt[:], self.sts.pop(m)[:], Exp)
                    self.pts[m] = pt

            def emit_acc(self, m):
                # VectorE: accumulate exp tiles for the softmax denominator
                if NOACC:
                    if m == 0:
                        nc.vector.memset(self.acc[:], 1.0)
                    return
                if m == 0:
                    nc.vector.tensor_copy(self.acc[:], self.pts[m][:])
                else:
                    nc.vector.tensor_tensor(
                        self.acc[:], self.acc[:], self.pts[m][:], AluOp.add
                    )

            def emit_pv_group(self, g):
                for m in range(g * STB, (g + 1) * STB):
                    ptm = dummy_pt if DUMMYPT else self.pts[m]
                    for j in range(NCH):
                        nc.tensor.matmul(
                            self.pv[j][:],
                            ptm[:, j * 128 : (j + 1) * 128],
                            mv_sb[:, m],
                            start=(m == 0),
                            stop=(m == MT - 1),
                        )
                    self.emit_acc(m)
                    del self.pts[m]

            def prologue(self):
                q = self.q
                self.pv = [
                    pvp.tile([128, NQS], f32, tag=f"pv{j}", name=f"pv_q{q}_{j}")
                    for j in range(NCH)
                ]
                self.acc = accp.tile([128, NQS], f32, tag="acc", name=f"acc_q{q}")
                for g in range(LEAD):
                    self.emit_st_exp_group(g)

            def mloop(self):
                for g in range(NG):
                    self.emit_pv_group(g)
                    if g + LEAD < NG:
                        self.emit_st_exp_group(g + LEAD)

            def epilogue(self):
                q = self.q
                # denominator: reduce acc over partitions with tiny matmuls;
                # output partition = n, which also gives the orientation the
                # per-partition normalize needs. The tile comes from the S^T
                # pool (drained by now) so all 4 psum banks stay with stp.
                dn = stp.tile([128, NQS], f32, tag="st", name=f"dn_q{q}")
                # cast the accumulator to bf16 so the partition-reduce matmuls
                # stay in the fast 1-cyc/row PE mode (fp32 lhsT would force
                # 4-cyc/row excursions mid-stream); den error ~0.4%/sqrt(128),
                # negligible
                accb = outp.tile([128, NQS], bf16, tag="accb", name=f"accb_q{q}")
                nc.vector.tensor_copy(accb[:], self.acc[:])
                for j in range(NCH):
                    nc.tensor.matmul(
                        dn[:, j : j + 1],
                        accb[:, j * 128 : (j + 1) * 128],
                        ones_sb[:],
                        start=True,
                        stop=True,
                    )
                recip = smallp.tile(
                    [128, NCH], f32, tag="recip", name=f"recip_q{q}"
                )
                nc.vector.reciprocal(recip[:], dn[:, :NCH])
                for j in range(NCH):
                    o = outp.tile([128, Cv], bf16, tag="o", name=f"o_q{q}_{j}")
                    # split the tail normalize across DVE and ACT so the
                    # final PSUM evacuation halves in wall-clock
                    if j % 2 == 0:
                        nc.vector.tensor_scalar_mul(
                            o[:], self.pv[j][:], recip[:, j : j + 1]
                        )
                    else:
                        nc.scalar.activation(
                            o[:], self.pv[j][:], Copy, scale=recip[:, j : j + 1]
                        )
                    n0 = q * NQS + j * 128
                    nc.scalar.dma_start(out_d[n0 : n0 + 128, :], o[:])

        # interleave: emit the next half's prologue before this half's
        # epilogue so PE isn't FIFO-blocked behind the denominator matmuls
        # (which wait on the DVE accumulation tail)
        halves = [Half(q) for q in range(NQ)]
        halves[0].prologue()
        for q in range(NQ):
            halves[q].mloop()
            if q + 1 < NQ:
                halves[q + 1].prologue()
            halves[q].epilogue()

    with tile.TileContext(nc) as tc:
        with (
            tc.tile_pool(name="big", bufs=1) as big,
            tc.tile_pool(name="inp", bufs=2) as inp,
            tc.tile_pool(name="ptp", bufs=PT_BUFS) as ptp,
            tc.tile_pool(name="accp", bufs=2) as accp,
            tc.tile_pool(name="outp", bufs=6) as outp,
            tc.tile_pool(name="smallp", bufs=4) as smallp,
            tc.tile_pool(name="stp", bufs=4, space="PSUM") as stp,
            tc.tile_pool(name="pvp", bufs=1, space="PSUM") as pvp,
        ):
            if mode == "compute" and loop > 1:
                tiles = emit_dma(nc, tc, big, inp)
                with tc.For_i(0, loop, 1):
                    body(nc, tc, big, ptp, accp, outp, smallp, stp, pvp, tiles)
            elif mode == "dma" and loop > 1:
                with tc.For_i(0, loop, 1):
                    emit_dma(nc, tc, big, inp)
                    for j in range(8):
                        o = outp.tile([128, Cv], f32, tag="o", name=f"o_{j}")
                        nc.vector.memset(o[:], float(j))
                        nc.sync.dma_start(out_d[j * 128 : (j + 1) * 128, :], o[:])
            else:
                loop_ctx = tc.For_i(0, loop, 1) if loop > 1 else None
                with (loop_ctx if loop_ctx is not None else _null()):
                    tiles = emit_dma(nc, tc, big, inp)
                    body(nc, tc, big, ptp, accp, outp, smallp, stp, pvp, tiles)

    nc.finalize()
    return nc


class _null:
    def __enter__(self):
        return None

    def __exit__(self, *a):
        return False


def _get_nc():
    key = ("nc", QK_MODE, LOOP, MODE, STB, LEAD, PT_BUFS, DUMMYPT, NOACC)
    if key not in _CACHE:
        _CACHE[key] = _build_nc(QK_MODE, LOOP, MODE)
    return _CACHE[key]


def _prep_core_inputs(memory_keys, memory_values, query_key, b):
    np_qk_dt = np.float16 if QK_MODE == "f16" else np.float32
    # astype on the transposed view fuses cast+copy in one pass
    qk = query_key[b].reshape(Ck, HW).astype(np_qk_dt)
    mk = memory_keys[:, b].transpose(1, 0, 2, 3).astype(np_qk_dt).reshape(Ck, M)
    mv = (
        memory_values[:, b]
        .transpose(0, 2, 3, 1)
        .astype(ml_dtypes.bfloat16)
        .reshape(M, Cv)
    )
    return {"qk": qk, "mk": mk, "mv": mv}


_RUNNER = {}


def _get_runner():
    """Build the sharded PJRT callable once and reuse it — the generic
    run_bass_kernel_spmd path re-traces jax.jit on every call (~2 s)."""
    if "r" not in _RUNNER:
        import jax
        from jax.sharding import Mesh, PartitionSpec, NamedSharding
        from jax.experimental.shard_map import shard_map

        import concourse.mybir as mybir
        from concourse import bass2jax
        from concourse.bass2jax import _bass_exec_p, install_neuronx_cc_hook

        nc = _get_nc()
        install_neuronx_cc_hook()
        pname = nc.partition_id_tensor.name if nc.partition_id_tensor else None
        in_names, out_names, out_avals = [], [], []
        for alloc in nc.m.functions[0].allocations:
            if not isinstance(alloc, mybir.MemoryLocationSet):
                continue
            name = alloc.memorylocations[0].name
            if alloc.kind == "ExternalInput":
                if name != pname:
                    in_names.append(name)
            elif alloc.kind == "ExternalOutput":
                out_names.append(name)
                out_avals.append(
                    jax.core.ShapedArray(
                        tuple(alloc.tensor_shape), mybir.dt.np(alloc.dtype)
                    )
                )
        n_params = len(in_names)
        all_in = list(in_names) + list(out_names) + ([pname] if pname else [])

        def _body(*args):
            operands = list(args)
            if pname is not None:
                operands.append(bass2jax.partition_id_tensor())
            return tuple(
                _bass_exec_p.bind(
                    *operands,
                    out_avals=tuple(out_avals),
                    in_names=tuple(all_in),
                    out_names=tuple(out_names),
                    lowering_input_output_aliases=(),
                    sim_require_finite=True,
                    sim_require_nnan=True,
                    nc=nc,
                )
            )

        mesh = Mesh(np.asarray(jax.devices()[:N_CORES]), ("core",))
        n_outs = len(out_names)
        sharded = jax.jit(
            shard_map(
                _body,
                mesh=mesh,
                in_specs=(PartitionSpec("core"),) * (n_params + n_outs),
                out_specs=(PartitionSpec("core"),) * n_outs,
                check_rep=False,
            ),
            keep_unused=True,
        )
        sh = NamedSharding(mesh, PartitionSpec("core"))
        zeros = [
            jax.device_put(
                np.zeros((N_CORES * a.shape[0], *a.shape[1:]), a.dtype), sh
            )
            for a in out_avals
        ]
        _RUNNER["r"] = (sharded, sh, in_names, zeros)
    return _RUNNER["r"]


def kernel(memory_keys, memory_values, query_key, query_value):
    global LAST_RESULTS
    memory_keys = np.asarray(memory_keys, dtype=np.float32)
    memory_values = np.asarray(memory_values, dtype=np.float32)
    query_key = np.asarray(query_key, dtype=np.float32)
    query_value = np.asarray(query_value, dtype=np.float32)

    in_maps = [
        _prep_core_inputs(memory_keys, memory_values, query_key, b)
        for b in range(N_CORES)
    ]
    try:
        import jax

        sharded, sh, in_names, zeros = _get_runner()
        dev_in = [
            jax.device_put(
                np.concatenate([in_maps[c][n] for c in range(N_CORES)], 0), sh
            )
            for n in in_names
        ]
        outs = sharded(*dev_in, *zeros)
        out_full = np.asarray(outs[0]).reshape(N_CORES, HW, Cv)
    except Exception:
        # conservative fallback: the generic (slower per call) exec path
        from concourse.bass_utils import run_bass_kernel_spmd

        res = run_bass_kernel_spmd(
            _get_nc(), in_maps, core_ids=list(range(N_CORES))
        )
        LAST_RESULTS = res
        out_full = np.stack([res.results[b]["out"] for b in range(N_CORES)])

    mem = np.stack(
        [out_full[b].T.reshape(Cv, H, W) for b in range(N_CORES)]
    ).astype(np.float32)
    return np.concatenate([query_value, mem], axis=1)
